# revision 37
# baseline (speedup 1.0000x reference)
"""BitLinear forward (ternary groupwise-quantized linear) on 8 Trainium2 NeuronCores.

Computation:  out = x @ ternary_quantize_groupwise(weight).T
  x: [2, 2048, 4096] f32, weight: [4096, 4096] f32, group=128 along in_features.

Sharding (tensor-parallel): weight rows (out_features) split across 8 cores
(512 rows each); x replicated; each core computes its [4096, 512] output
slice; host concatenates along the feature dim.

Kernel design (single-pass f16, quant/matmul/DMA overlap), measured
~305 us on HW vs the 585 us dual-pass baseline:
  - x ships f16, host-pre-transposed to [D, T] (rel err ~3e-4 vs the 2e-2
    gate; the baseline's dual hi/lo passes are unnecessary and double the
    PE work).
  - w shard quantized on-chip in [o-part, d-free] layout (per-group f32
    absmean scale, exact threshold compare), f16-cast, DMA-xbar transposed
    into wqT [d-part, o-free].
  - flipped matmul roles: the quantized weight tile [128d, 128o] is the PE
    stationary operand, x [128d, 512t] streams; LDWEIGHTS hides fully
    under the 512-wide moving stream. PSUM holds [o, t]; output stores
    transposed [O_shard, T] f16, fixed up on the host.
  - tokens processed in two 2048-token halves sharing ONE SBUF buffer;
    subtile deps give per-block WAR so half 1's x streams in exactly as
    half 0's last pass finishes reading each block. Halves (not smaller
    spans) because the o-tile pass cadence (~28 us) must stay above the
    w-load-limited quant cadence (~15 us/o-tile) or the first pass-set
    stalls on wqT.
  - each DMA queue sustains only ~140 GB/s and the HWDGE ring queues are
    ~4 deep (excess issues block the issuing ENGINE), so bulk traffic is
    spread: w (8.4 MB, 1 MB chunks) exclusively on gpsimd; x half-0 as
    eight 2 MB 4-group blocks, evens on scalar / odds on sync, saturating
    both rings from t=0 (the first transpose slips ~2 block-transfers but
    x completes much sooner, which measured strictly better than weaving
    x behind the transposes); x half-1 alternates gpsimd/scalar; wq
    transposes and output stores on sync.
"""

from contextlib import ExitStack

import numpy as np

import concourse.bacc as bacc
import concourse.mybir as mybir
import concourse.tile as tile

# Problem shapes (hardcoded per contract; kernel.py must be self-contained).
B, S, DIM_D, DIM_O = 2, 2048, 4096, 4096
T = B * S                 # 4096 tokens
NCORES = 8
O_SHARD = DIM_O // NCORES  # 512 out features per core
P = 128                    # SBUF partitions / PE array dim
GROUP = 128                # quant group size along in_features
EPS = 1e-8

f32 = mybir.dt.float32
f16 = mybir.dt.float16

DEFAULT_CFG = dict(
    q_chunks=4,        # quant compute chunks per o-tile (pipeline grain)
    w_chunks=2,        # w DMA chunks per o-tile (8KB rows/packets)
    t_span=2048,       # token span size (xq double-buffered)
    nblk=512,          # moving/psum free size (one PSUM bank of f32)
    psum_bufs=8,
    osb_bufs=6,
    out_dtype="float16",
)


def _emit(ctx, tc, xh, w, out, T_, D_, O_, cfg):
    """Per-core program. xh: [D_, T_] f16 DRAM; w: [O_, D_] f32 DRAM;
    out: [O_, T_] f16 DRAM (transposed output)."""
    nc = tc.nc
    G = D_ // P                # 32 groups along D
    OT = O_ // P               # 4 o-tiles
    QCH = cfg["q_chunks"]
    DC = D_ // QCH             # quant compute chunk width
    GC = G // QCH              # groups per compute chunk
    TS = cfg["t_span"]
    NS = T_ // TS              # spans
    NBLK = cfg["nblk"]
    TB = TS // NBLK            # token blocks (psum banks) per pass
    out_dt = getattr(mybir.dt, cfg["out_dtype"])
    WCH = cfg["w_chunks"]
    WDC = D_ // WCH
    HPW = QCH // WCH           # compute chunks per w DMA chunk

    sb = ctx.enter_context(tc.tile_pool(name="sb", bufs=1))
    psum = ctx.enter_context(
        tc.tile_pool(name="psum", bufs=cfg["psum_bufs"], space="PSUM"))

    wqT = sb.tile([P, G, O_], f16, tag="wqT", bufs=1)
    # ONE x buffer, spans overwrite it via subregion DMAs: subtile deps give
    # per-block WAR, so span s+1's blocks stream in as soon as span s's last
    # pass has read each block (a second tile on the same pool slot would
    # make the WAR whole-tile and serialize the halves).
    xsb = sb.tile([P, G, TS], f16, tag="xsb", bufs=1)
    xq = [xsb for _ in range(NS)]

    # x loads as 4-g block DMAs (1 MB each): the HWDGE ring queues are only
    # ~4 deep — more outstanding issues block the issuing ENGINE, so keep
    # at most ~2 in flight per ring and weave the rest.
    XB = 4                     # g's per x block DMA
    NXB = G // XB              # 8 blocks per span

    def x_block(eng, s, b):
        gsl = slice(b * XB, (b + 1) * XB)
        return eng.dma_start(
            xq[s][:, gsl, :],
            xh[b * XB * P:(b + 1) * XB * P, s * TS:(s + 1) * TS].rearrange(
                "(j p) t -> p j t", p=P))

    # ---- Phase Q: quantize w shard o-tile by o-tile, producing wqT.
    # span-0 x: even blocks on scalar, odd blocks on sync.
    x0_scalar = iter(range(0, NXB, 2))
    x0_sync = iter(range(1, NXB, 2))

    def issue_x0_scalar(n):
        for _ in range(n):
            b = next(x0_scalar, None)
            if b is not None:
                x_block(nc.scalar, 0, b)

    def issue_x0_sync(n):
        for _ in range(n):
            b = next(x0_sync, None)
            if b is not None:
                x_block(nc.sync, 0, b)

    # saturate both rings immediately (before the transposes are ready);
    # the first transpose slips behind ~2 block transfers but x span 0
    # completes much sooner overall, which measured strictly better than
    # weaving x behind the transposes.
    issue_x0_scalar(2)
    issue_x0_sync(2)

    # all w DMAs upfront on gpsimd; wt double-buffer paces them naturally
    wt_tiles = {}
    for ot in range(OT):
        for wh in range(WCH):
            wt = sb.tile([P, WDC], f32, tag="wt", bufs=2, name=f"wt{ot}_{wh}")
            nc.gpsimd.dma_start(
                wt[:], w[ot * P:(ot + 1) * P, wh * WDC:(wh + 1) * WDC])
            wt_tiles[(ot, wh)] = wt

    last_transpose = None
    for ot in range(OT):
        for h in range(QCH):
            wt = wt_tiles[(ot, h // HPW)]
            wtv = wt[:, (h % HPW) * DC:(h % HPW + 1) * DC]

            abs_w = sb.tile([P, DC], f32, tag="abs_w", bufs=2)
            nc.scalar.activation(abs_w[:], wtv, mybir.ActivationFunctionType.Abs)
            sgn = sb.tile([P, DC], f16, tag="sgn", bufs=2)
            nc.scalar.activation(sgn[:], wtv, mybir.ActivationFunctionType.Sign)

            red = sb.tile([P, GC], f32, tag="red", bufs=2)
            nc.vector.tensor_reduce(
                red[:], abs_w[:].rearrange("p (g j) -> p g j", j=GROUP),
                axis=mybir.AxisListType.X, op=mybir.AluOpType.add,
            )
            # thr = 0.5*max(red/128, EPS) = max(red/256, EPS/2) (exact in f32)
            thr = sb.tile([P, GC], f32, tag="thr", bufs=2)
            nc.vector.tensor_scalar(
                thr[:], red[:], 1.0 / 256.0, EPS / 2.0,
                op0=mybir.AluOpType.mult, op1=mybir.AluOpType.max,
            )
            # scale rounded to f16 (the only weight-side precision loss)
            s16 = sb.tile([P, GC], f16, tag="s16", bufs=2)
            nc.vector.tensor_scalar(
                s16[:], red[:], 1.0 / 128.0, EPS,
                op0=mybir.AluOpType.mult, op1=mybir.AluOpType.max,
            )
            # c = (|w| > thr); q = c*sign(w); wq = q*scale16 (broadcasts via
            # stride-0 APs; no materialized scale rows)
            c = sb.tile([P, DC], f16, tag="c", bufs=2)
            nc.vector.tensor_tensor(
                c[:].rearrange("p (g j) -> p g j", j=GROUP),
                abs_w[:].rearrange("p (g j) -> p g j", j=GROUP),
                thr[:].unsqueeze(2).broadcast_to((P, GC, GROUP)),
                op=mybir.AluOpType.is_gt,
            )
            q = sb.tile([P, DC], f16, tag="q", bufs=2)
            nc.vector.tensor_tensor(q[:], c[:], sgn[:], op=mybir.AluOpType.mult)
            wq = sb.tile([P, DC], f16, tag="wq", bufs=2)
            nc.vector.tensor_tensor(
                wq[:].rearrange("p (g j) -> p g j", j=GROUP),
                q[:].rearrange("p (g j) -> p g j", j=GROUP),
                s16[:].unsqueeze(2).broadcast_to((P, GC, GROUP)),
                op=mybir.AluOpType.mult,
            )
            # wqT[p, h*GC+a, ot*P+b] = wq[b, a*P+p]; all transposes stay on
            # the sync ring (two-ring transposes corrupt data on HW).
            t_inst = nc.sync.dma_start_transpose(
                wqT[:, h * GC:(h + 1) * GC, ot * P:(ot + 1) * P], wq[:])
            last_transpose = t_inst
            # remaining span-0 x blocks paced through the early chunks
            ch = ot * QCH + h
            if ch in (0, 2):
                issue_x0_scalar(1)
            elif ch in (1, 3):
                issue_x0_sync(1)
    issue_x0_scalar(NXB)
    issue_x0_sync(NXB)

    # ---- Phase M: matmul passes. Span s+1 x loads are emitted after span
    # s's matmuls (dep tracking orders only against already-emitted
    # readers); evens on scalar, odds on gpsimd (idle once w is loaded).
    # WAR on the 2-deep xq buffer ring is tracked by the pool.
    for s in range(NS):
        tbase = s * TS
        for ot2 in range(OT):
            osl = slice(ot2 * P, (ot2 + 1) * P)
            ps = [psum.tile([P, NBLK], f32, tag="ps", name=f"ps{s}_{ot2}_{tb}")
                  for tb in range(TB)]
            for g in range(G):
                for tb in range(TB):
                    nc.tensor.matmul(
                        ps[tb][:], lhsT=wqT[:, g, osl],
                        rhs=xq[s][:, g, tb * NBLK:(tb + 1) * NBLK],
                        start=(g == 0), stop=(g == G - 1),
                    )
            for tb in range(TB):
                osb = sb.tile([P, NBLK], out_dt, tag="osb", bufs=cfg["osb_bufs"])
                nc.scalar.copy(osb[:], ps[tb][:])
                nc.sync.dma_start(
                    out[osl, tbase + tb * NBLK:tbase + (tb + 1) * NBLK], osb[:])
        # span s+1 x loads, emitted AFTER span s's matmuls so the subtile
        # WAR orders each block behind span s's last reader of that block
        # (emitting earlier races the overwrite against span s's reads)
        if s + 1 < NS:
            for b in range(NXB):
                eng = nc.gpsimd if b % 2 == 0 else nc.scalar
                x_block(eng, s + 1, b)


def build_nc(T_=T, D_=DIM_D, O_=O_SHARD, cfg=None):
    cfg = {**DEFAULT_CFG, **(cfg or {})}
    nc = bacc.Bacc("TRN2", target_bir_lowering=False, debug=False)
    xh = nc.declare_dram_parameter("xh", [D_, T_], f16, isOutput=False)
    w = nc.declare_dram_parameter("w", [O_, D_], f32, isOutput=False)
    out_dt = getattr(mybir.dt, cfg["out_dtype"])
    out = nc.declare_dram_parameter("out", [O_, T_], out_dt, isOutput=True)
    with tile.TileContext(nc) as tc:
        with ExitStack() as ctx:
            _emit(ctx, tc, xh.ap(), w.ap(), out.ap(), T_, D_, O_, cfg)
    nc.compile()
    return nc


def prepare_inputs(x, weight):
    xf = np.ascontiguousarray(np.asarray(x, dtype=np.float32).reshape(T, DIM_D))
    wf = np.ascontiguousarray(np.asarray(weight, dtype=np.float32))
    xh = np.ascontiguousarray(xf.astype(np.float16).T)
    in_maps = []
    for c in range(NCORES):
        in_maps.append({
            "xh": xh,
            "w": np.ascontiguousarray(wf[c * O_SHARD:(c + 1) * O_SHARD]),
        })
    return in_maps


def run(x, weight, trace=False, cfg=None, **kwargs):
    from concourse.bass_utils import run_bass_kernel_spmd

    nc = build_nc(cfg=cfg)
    in_maps = prepare_inputs(x, weight)
    res = run_bass_kernel_spmd(
        nc, in_maps, core_ids=list(range(NCORES)), trace=trace, **kwargs
    )
    # out is [O_shard, T] per core; transpose + concat along features
    outs = [np.asarray(res.results[c]["out"]).astype(np.float32).T
            for c in range(NCORES)]
    full = np.concatenate(outs, axis=1).reshape(B, S, DIM_O)
    return full, res


def kernel(x, weight):
    full, _ = run(x, weight, trace=False)
    return full.astype(np.float32)


# revision 38
# speedup vs baseline: 1.1567x; 1.1567x over previous
"""BitLinear forward (ternary groupwise-quantized linear) on 8 Trainium2 NeuronCores.

Computation:  out = x @ ternary_quantize_groupwise(weight).T
  x: [2, 2048, 4096] f32, weight: [4096, 4096] f32, group=128 along in_features.

Sharding (tensor-parallel): weight rows (out_features) split across 8 cores
(512 rows each); x replicated; each core computes its [4096, 512] output
slice; host concatenates along the feature dim.

Kernel design (single-pass f16, quant/matmul/DMA overlap), measured
~305 us on HW vs the 585 us dual-pass baseline:
  - x ships f16, host-pre-transposed to [D, T] (rel err ~3e-4 vs the 2e-2
    gate; the baseline's dual hi/lo passes are unnecessary and double the
    PE work).
  - w shard quantized on-chip in [o-part, d-free] layout (per-group f32
    absmean scale, exact threshold compare), f16-cast, DMA-xbar transposed
    into wqT [d-part, o-free].
  - flipped matmul roles: the quantized weight tile [128d, 128o] is the PE
    stationary operand, x [128d, 512t] streams; LDWEIGHTS hides fully
    under the 512-wide moving stream. PSUM holds [o, t]; output stores
    transposed [O_shard, T] f16, fixed up on the host.
  - tokens processed in two 2048-token halves sharing ONE SBUF buffer;
    subtile deps give per-block WAR so half 1's x streams in exactly as
    half 0's last pass finishes reading each block. Halves (not smaller
    spans) because the o-tile pass cadence (~28 us) must stay above the
    w-load-limited quant cadence (~15 us/o-tile) or the first pass-set
    stalls on wqT.
  - each DMA queue sustains only ~140 GB/s and the HWDGE ring queues are
    ~4 deep (excess issues block the issuing ENGINE), so bulk traffic is
    spread: w (8.4 MB, 1 MB chunks) exclusively on gpsimd; x half-0 as
    eight 2 MB 4-group blocks, evens on scalar / odds on sync, saturating
    both rings from t=0 (the first transpose slips ~2 block-transfers but
    x completes much sooner, which measured strictly better than weaving
    x behind the transposes); x half-1 alternates gpsimd/scalar; wq
    transposes and output stores on sync.
"""

from contextlib import ExitStack

import numpy as np

import concourse.bacc as bacc
import concourse.mybir as mybir
import concourse.tile as tile

# Problem shapes (hardcoded per contract; kernel.py must be self-contained).
B, S, DIM_D, DIM_O = 2, 2048, 4096, 4096
T = B * S                 # 4096 tokens
NCORES = 8
O_SHARD = DIM_O // NCORES  # 512 out features per core
P = 128                    # SBUF partitions / PE array dim
GROUP = 128                # quant group size along in_features
EPS = 1e-8

f32 = mybir.dt.float32
f16 = mybir.dt.float16

DEFAULT_CFG = dict(
    q_chunks=4,        # quant compute chunks per o-tile (pipeline grain)
    w_chunks=2,        # w DMA chunks per o-tile (8KB rows/packets)
    t_span=2048,       # token span size (xq double-buffered)
    nblk=512,          # moving/psum free size (one PSUM bank of f32)
    psum_bufs=8,
    osb_bufs=6,
    out_dtype="float16",
)


def _emit(ctx, tc, xh, w, out, T_, D_, O_, cfg):
    """Per-core program. xh: [D_, T_] f16 DRAM; w: [O_, D_] f32 DRAM;
    out: [O_, T_] f16 DRAM (transposed output)."""
    nc = tc.nc
    G = D_ // P                # 32 groups along D
    OT = O_ // P               # 4 o-tiles
    QCH = cfg["q_chunks"]
    DC = D_ // QCH             # quant compute chunk width
    GC = G // QCH              # groups per compute chunk
    TS = cfg["t_span"]
    NS = T_ // TS              # spans
    NBLK = cfg["nblk"]
    TB = TS // NBLK            # token blocks (psum banks) per pass
    out_dt = getattr(mybir.dt, cfg["out_dtype"])
    WCH = cfg["w_chunks"]
    WDC = D_ // WCH
    HPW = QCH // WCH           # compute chunks per w DMA chunk

    sb = ctx.enter_context(tc.tile_pool(name="sb", bufs=1))
    psum = ctx.enter_context(
        tc.tile_pool(name="psum", bufs=cfg["psum_bufs"], space="PSUM"))

    wqT = sb.tile([P, G, O_], f16, tag="wqT", bufs=1)
    # ONE x buffer, spans overwrite it via subregion DMAs: subtile deps give
    # per-block WAR, so span s+1's blocks stream in as soon as span s's last
    # pass has read each block (a second tile on the same pool slot would
    # make the WAR whole-tile and serialize the halves).
    xsb = sb.tile([P, G, TS], f16, tag="xsb", bufs=1)
    xq = [xsb for _ in range(NS)]

    # x loads as 4-g block DMAs (1 MB each): the HWDGE ring queues are only
    # ~4 deep — more outstanding issues block the issuing ENGINE, so keep
    # at most ~2 in flight per ring and weave the rest.
    XB = 4                     # g's per x block DMA
    NXB = G // XB              # 8 blocks per span

    def x_block(eng, s, b):
        gsl = slice(b * XB, (b + 1) * XB)
        return eng.dma_start(
            xq[s][:, gsl, :],
            xh[b * XB * P:(b + 1) * XB * P, s * TS:(s + 1) * TS].rearrange(
                "(j p) t -> p j t", p=P))

    # ---- Phase Q: quantize w shard o-tile by o-tile, producing wqT.
    # span-0 x: even blocks on scalar, odd blocks on sync.
    x0_scalar = iter(range(0, NXB, 2))
    x0_sync = iter(range(1, NXB, 2))

    def issue_x0_scalar(n):
        for _ in range(n):
            b = next(x0_scalar, None)
            if b is not None:
                x_block(nc.scalar, 0, b)

    def issue_x0_sync(n):
        for _ in range(n):
            b = next(x0_sync, None)
            if b is not None:
                x_block(nc.sync, 0, b)

    # saturate both rings immediately (before the transposes are ready);
    # the first transpose slips behind ~2 block transfers but x span 0
    # completes much sooner overall, which measured strictly better than
    # weaving x behind the transposes.
    issue_x0_scalar(2)
    issue_x0_sync(1)

    # all w DMAs upfront on gpsimd; wt double-buffer paces them naturally
    wt_tiles = {}
    for ot in range(OT):
        for wh in range(WCH):
            wt = sb.tile([P, WDC], f32, tag="wt", bufs=2, name=f"wt{ot}_{wh}")
            nc.gpsimd.dma_start(
                wt[:], w[ot * P:(ot + 1) * P, wh * WDC:(wh + 1) * WDC])
            wt_tiles[(ot, wh)] = wt

    last_transpose = None
    for ot in range(OT):
        for h in range(QCH):
            wt = wt_tiles[(ot, h // HPW)]
            wtv = wt[:, (h % HPW) * DC:(h % HPW + 1) * DC]

            abs_w = sb.tile([P, DC], f32, tag="abs_w", bufs=2)
            nc.scalar.activation(abs_w[:], wtv, mybir.ActivationFunctionType.Abs)
            sgn = sb.tile([P, DC], f16, tag="sgn", bufs=2)
            nc.scalar.activation(sgn[:], wtv, mybir.ActivationFunctionType.Sign)

            red = sb.tile([P, GC], f32, tag="red", bufs=2)
            nc.vector.tensor_reduce(
                red[:], abs_w[:].rearrange("p (g j) -> p g j", j=GROUP),
                axis=mybir.AxisListType.X, op=mybir.AluOpType.add,
            )
            # thr = 0.5*max(red/128, EPS) = max(red/256, EPS/2) (exact in f32)
            thr = sb.tile([P, GC], f32, tag="thr", bufs=2)
            nc.vector.tensor_scalar(
                thr[:], red[:], 1.0 / 256.0, EPS / 2.0,
                op0=mybir.AluOpType.mult, op1=mybir.AluOpType.max,
            )
            # scale rounded to f16 (the only weight-side precision loss)
            s16 = sb.tile([P, GC], f16, tag="s16", bufs=2)
            nc.vector.tensor_scalar(
                s16[:], red[:], 1.0 / 128.0, EPS,
                op0=mybir.AluOpType.mult, op1=mybir.AluOpType.max,
            )
            # c = (|w| > thr); q = c*sign(w); wq = q*scale16 (broadcasts via
            # stride-0 APs; no materialized scale rows)
            c = sb.tile([P, DC], f16, tag="c", bufs=2)
            nc.vector.tensor_tensor(
                c[:].rearrange("p (g j) -> p g j", j=GROUP),
                abs_w[:].rearrange("p (g j) -> p g j", j=GROUP),
                thr[:].unsqueeze(2).broadcast_to((P, GC, GROUP)),
                op=mybir.AluOpType.is_gt,
            )
            q = sb.tile([P, DC], f16, tag="q", bufs=2)
            nc.vector.tensor_tensor(q[:], c[:], sgn[:], op=mybir.AluOpType.mult)
            wq = sb.tile([P, DC], f16, tag="wq", bufs=2)
            nc.vector.tensor_tensor(
                wq[:].rearrange("p (g j) -> p g j", j=GROUP),
                q[:].rearrange("p (g j) -> p g j", j=GROUP),
                s16[:].unsqueeze(2).broadcast_to((P, GC, GROUP)),
                op=mybir.AluOpType.mult,
            )
            # wqT[p, h*GC+a, ot*P+b] = wq[b, a*P+p]; all transposes stay on
            # the sync ring (two-ring transposes corrupt data on HW).
            with tc.high_priority():
                t_inst = nc.sync.dma_start_transpose(
                    wqT[:, h * GC:(h + 1) * GC, ot * P:(ot + 1) * P], wq[:])
            last_transpose = t_inst
            # remaining span-0 x blocks paced through the early chunks
            ch = ot * QCH + h
            if ch in (0, 2):
                issue_x0_scalar(1)
                issue_x0_sync(1)
            elif ch in (1, 3):
                issue_x0_sync(1)
    issue_x0_scalar(NXB)
    issue_x0_sync(NXB)

    # ---- Phase M: matmul passes. Span s+1 x loads are emitted after span
    # s's matmuls (dep tracking orders only against already-emitted
    # readers); evens on scalar, odds on gpsimd (idle once w is loaded).
    # WAR on the 2-deep xq buffer ring is tracked by the pool.
    for s in range(NS):
        tbase = s * TS
        for ot2 in range(OT):
            osl = slice(ot2 * P, (ot2 + 1) * P)
            ps = [psum.tile([P, NBLK], f32, tag="ps", name=f"ps{s}_{ot2}_{tb}")
                  for tb in range(TB)]
            for g in range(G):
                for tb in range(TB):
                    nc.tensor.matmul(
                        ps[tb][:], lhsT=wqT[:, g, osl],
                        rhs=xq[s][:, g, tb * NBLK:(tb + 1) * NBLK],
                        start=(g == 0), stop=(g == G - 1),
                    )
            for tb in range(TB):
                osb = sb.tile([P, NBLK], out_dt, tag="osb", bufs=cfg["osb_bufs"])
                nc.scalar.copy(osb[:], ps[tb][:])
                nc.sync.dma_start(
                    out[osl, tbase + tb * NBLK:tbase + (tb + 1) * NBLK], osb[:])
        # span s+1 x loads, emitted AFTER span s's matmuls so the subtile
        # WAR orders each block behind span s's last reader of that block
        # (emitting earlier races the overwrite against span s's reads)
        if s + 1 < NS:
            for b in range(NXB):
                eng = nc.gpsimd if b % 2 == 0 else nc.scalar
                x_block(eng, s + 1, b)


def build_nc(T_=T, D_=DIM_D, O_=O_SHARD, cfg=None):
    cfg = {**DEFAULT_CFG, **(cfg or {})}
    nc = bacc.Bacc("TRN2", target_bir_lowering=False, debug=False)
    xh = nc.declare_dram_parameter("xh", [D_, T_], f16, isOutput=False)
    w = nc.declare_dram_parameter("w", [O_, D_], f32, isOutput=False)
    out_dt = getattr(mybir.dt, cfg["out_dtype"])
    out = nc.declare_dram_parameter("out", [O_, T_], out_dt, isOutput=True)
    with tile.TileContext(nc) as tc:
        with ExitStack() as ctx:
            _emit(ctx, tc, xh.ap(), w.ap(), out.ap(), T_, D_, O_, cfg)
    nc.compile()
    return nc


def prepare_inputs(x, weight):
    xf = np.ascontiguousarray(np.asarray(x, dtype=np.float32).reshape(T, DIM_D))
    wf = np.ascontiguousarray(np.asarray(weight, dtype=np.float32))
    xh = np.ascontiguousarray(xf.astype(np.float16).T)
    in_maps = []
    for c in range(NCORES):
        in_maps.append({
            "xh": xh,
            "w": np.ascontiguousarray(wf[c * O_SHARD:(c + 1) * O_SHARD]),
        })
    return in_maps


def run(x, weight, trace=False, cfg=None, **kwargs):
    from concourse.bass_utils import run_bass_kernel_spmd

    nc = build_nc(cfg=cfg)
    in_maps = prepare_inputs(x, weight)
    res = run_bass_kernel_spmd(
        nc, in_maps, core_ids=list(range(NCORES)), trace=trace, **kwargs
    )
    # out is [O_shard, T] per core; transpose + concat along features
    outs = [np.asarray(res.results[c]["out"]).astype(np.float32).T
            for c in range(NCORES)]
    full = np.concatenate(outs, axis=1).reshape(B, S, DIM_O)
    return full, res


def kernel(x, weight):
    full, _ = run(x, weight, trace=False)
    return full.astype(np.float32)
